# revision 30
# baseline (speedup 1.0000x reference)
"""TRN2 Bass kernel for nn_Encoder_27290222198965.

Reference computation (N=8, L=2048, H=1024):
    q = x@Wq.T+bq ; k = x@Wk.T+bk ; v = x@Wv.T+bv
    d[n,l] = sum_h q*k                       (diagonal "attention" scores)
    att = softmax(diag-embed(d), axis=2) ->  colsum[n,l] = S[n] + (e-1)/(L-1+e),
        e = exp(d[n,l]), S[n] = sum_l 1/(L-1+exp(d[n,l]))
    out = (colsum[:, :, None] * v) @ Wo.T + bo

Algebraic refactor (validated to ~4e-6 rel err with exact matmuls):
    d[n,l] = rowsum(x ⊙ y') + c0,  y' = x @ M^T + u,
        M = Wq^T Wk, u = Wk^T bq + Wq^T bk, c0 = bq·bk
    colsum = (S+1) - 2048*r,  r = 1/(2047+exp(d)),  S = sum_l r
        (uses e*r = 1 - 2047*r)
    out    = colsum ⊙ (x @ Wc^T + bc) + bo,  Wc = Wo@Wv, bc = Wo@bv
so only TWO HxH projections run on hardware (y' and z) instead of four.

Sharding: data-parallel over N — core n handles batch n. All matmuls in
float32r (full PE rate at free dim 512, ~e8m12 effective precision).
Everything on-chip is transposed ([feature, token]) so biases are
per-partition and fold into ScalarE psum->sbuf copies. DMA is emitted in
just-in-time consumption order (per-ob weight tiles, per-hb x tiles) so the
PE starts within ~2us of kernel start.
"""

import numpy as np

import concourse.bass as bass  # noqa: F401  (registers engines on Bacc)
import concourse.tile as tile
from concourse import bacc, mybir
from concourse.bass_utils import run_bass_kernel_spmd

dt = mybir.dt
AF = mybir.ActivationFunctionType
ALU = mybir.AluOpType

N, L, H = 8, 2048, 1024
P = 128            # SBUF partitions
LB = 512           # l-block (moving free dim of every matmul)
NH = H // P        # 8 h-blocks
NL = L // LB       # 4 l-blocks
N_CORES = 8

_CACHE = {}


def _build():
    nc = bacc.Bacc("TRN2", target_bir_lowering=False, debug=False,
                   num_devices=N_CORES)

    xT_d = nc.dram_tensor("xT", [H, L], dt.float32r, kind="ExternalInput").ap()
    MT_d = nc.dram_tensor("MT", [NH, P, NH * P], dt.float32r, kind="ExternalInput").ap()
    WcT_d = nc.dram_tensor("WcT", [NH, P, NH * P], dt.float32r, kind="ExternalInput").ap()
    cp_d = nc.dram_tensor("cpack", [P, NH + 1 + P], dt.float32r,
                          kind="ExternalInput").ap()
    bcb_d = nc.dram_tensor("bcb", [P, NH], dt.float32, kind="ExternalInput").ap()
    bob_d = nc.dram_tensor("bob", [P, NH], dt.float32, kind="ExternalInput").ap()
    out_d = nc.dram_tensor("outT", [H, L], dt.float32, kind="ExternalOutput").ap()

    xT3 = xT_d.rearrange("(j p) l -> p j l", p=P)    # [128, 8, 2048]
    MT3 = MT_d    # prepacked [ob, p(hin%128), hb*128+hout]
    WcT3 = WcT_d

    with tile.TileContext(nc) as tc:
        with (
            tc.tile_pool(name="resident", bufs=1) as rp,
            tc.tile_pool(name="weights", bufs=1) as wtp,
            tc.tile_pool(name="xstream", bufs=24) as xp,
            tc.tile_pool(name="work", bufs=3) as wp,
            tc.tile_pool(name="mmpsum", bufs=4, space="PSUM") as yp,
            tc.tile_pool(name="dpsum", bufs=2, space="PSUM") as dp,
        ):
            t_s = rp.tile([P, L], dt.float32)
            cs = rp.tile([P, L], dt.float32)

            def load_w(src3, ob, tag, eng=None):
                """One per-ob weight tile [hin(P), hb*P+hout] = 512KB."""
                t = wtp.tile([P, NH * P], dt.float32r, tag=f"{tag}{ob}")
                (eng or nc.sync).dma_start(t[:], src3[ob])
                return t

            def load_xb(lb, hb, eng=None):
                t = xp.tile([P, LB], dt.float32r, tag="xb")
                (eng or nc.sync).dma_start(
                    t[:], xT3[:, hb, lb * LB:(lb + 1) * LB])
                return t

            # ---- JIT DMA emission for the cold start ----
            mt = [None] * NH
            wct_holder = [None] * NH
            consts = {}
            xbs = {}
            cp = rp.tile([P, NH + 1 + P], dt.float32r)
            mt[0] = load_w(MT3, 0, "mt", eng=nc.scalar)
            for hb in range(NH):
                xbs[(0, hb)] = load_xb(0, hb)
                if hb == 3:  # consts mid-stream: needed only from t~15us on
                    nc.sync.dma_start(cp[:], cp_d[:])
            ub = cp[:, :NH].bitcast(dt.float32)
            c0b = cp[:, NH:NH + 1].bitcast(dt.float32)
            ones = cp[:, NH + 1:]
            for ob in range(1, NH):
                mt[ob] = load_w(MT3, ob, "mt", eng=nc.scalar)
            for hb in range(NH):
                xbs[(1, hb)] = load_xb(1, hb)

            # d-matmul bookkeeping: delay each block's last rowsum-MM into the
            # next MM group so the PE never waits on the ACT->DVE prod chain.
            state = {"pending": None}

            def flush_pending():
                if state["pending"] is None:
                    return
                pd_t, ob, prod_t, is_last, lb = state["pending"]
                nc.tensor.matmul(pd_t[:], ones, prod_t[:],
                                 start=(ob == 0), stop=is_last)
                state["pending"] = None
                if is_last:
                    # t = sigmoid(-d - c0 + ln(L-1)); r = t/(L-1)
                    # (1/((L-1)+e^d) = sigmoid(-d+ln(L-1))/(L-1))
                    ls = slice(lb * LB, (lb + 1) * LB)
                    nc.scalar.activation(t_s[:, ls], pd_t[:], AF.Sigmoid,
                                         bias=c0b[:, 0:1], scale=-1.0)

            # ================= phase 1: y' ; d ; r ==================
            for lb in range(NL):
                pd = dp.tile([P, LB], dt.float32)
                acc = None
                for ob in range(NH):
                    py = yp.tile([P, LB], dt.float32, tag="mm")
                    for hb in range(NH):
                        nc.tensor.matmul(
                            py[:], mt[ob][:, hb * P:(hb + 1) * P],
                            xbs[(lb, hb)][:],
                            start=(hb == 0), stop=(hb == NH - 1))
                    if ob == 1:
                        flush_pending()
                    yb = wp.tile([P, LB], dt.float32, tag="yb")
                    nc.scalar.activation(yb[:], py[:], AF.Identity,
                                         bias=ub[:, ob:ob + 1], scale=1.0)
                    prod = wp.tile([P, LB], dt.float32r, tag="prod")
                    nc.vector.tensor_tensor(
                        prod[:], yb[:], xbs[(lb, ob)][:].bitcast(dt.float32),
                        op=ALU.mult)
                    if acc is None:
                        acc = prod
                    else:
                        nacc = wp.tile([P, LB], dt.float32r, tag="pacc")
                        nc.vector.tensor_tensor(nacc[:], acc[:], prod[:],
                                                op=ALU.add)
                        acc = nacc
                state["pending"] = (pd, 0, acc, True, lb)
                # prefetch x for block lb+2 of phase 1, or re-reads for phase 2
                nxt = lb + 2
                if nxt < NL:
                    for hb in range(NH):
                        xbs[(nxt, hb)] = load_xb(nxt, hb)
                elif nxt == NL:  # after block 2: phase-2 weights
                    consts["bcb"] = rp.tile([P, NH], dt.float32, name="bcbt", tag="bcb")
                    nc.sync.dma_start(consts["bcb"][:], bcb_d[:])
                    consts["bob"] = rp.tile([P, NH], dt.float32, name="bobt", tag="bob")
                    nc.sync.dma_start(consts["bob"][:], bob_d[:])
                    for ob in range(NH):
                        wct_holder[ob] = load_w(WcT3, ob, "wct")
                else:            # after block 3: phase-2 x block 0
                    for hb in range(NH):
                        xbs[("p2", 0, hb)] = load_xb(0, hb)

            # ================= phase 2: z ; out ==================
            for lb in range(NL):
                for ob in range(NH):
                    last_grp = (lb == NL - 1 and ob == NH - 1)
                    nmm = 2 if last_grp else 1
                    mw = LB // nmm
                    pzs = []
                    for ck in range(nmm):
                        pz = yp.tile([P, mw], dt.float32, tag="mm")
                        for hb in range(NH):
                            nc.tensor.matmul(
                                pz[:], wct_holder[ob][:, hb * P:(hb + 1) * P],
                                xbs[("p2", lb, hb)][:, ck * mw:(ck + 1) * mw],
                                start=(hb == 0), stop=(hb == NH - 1))
                        pzs.append(pz)
                    if lb == 0 and ob == 0:
                        flush_pending()   # last d-MM of phase 1
                        # colsum = (1 + sum(t)/(L-1)) - (L/(L-1))*t
                        S_t = rp.tile([P, 1], dt.float32)
                        nc.vector.tensor_reduce(
                            S_t[:], t_s[:], axis=mybir.AxisListType.X,
                            op=ALU.add)
                        S1_t = rp.tile([P, 1], dt.float32)
                        nc.vector.tensor_scalar(
                            S1_t[:], S_t[:], 1.0 / (L - 1), 1.0,
                            op0=ALU.mult, op1=ALU.add)
                        nc.vector.tensor_scalar(
                            cs[:], t_s[:], -float(L) / (L - 1), S1_t[:],
                            op0=ALU.mult, op1=ALU.add)
                    # final group: half-width chunks so the last out-DMA
                    # starts ~1.3us earlier (shorter kernel tail)
                    nchunk = nmm
                    cw = LB // nchunk
                    for ck in range(nchunk):
                        lo = lb * LB + ck * cw
                        lsc = slice(lo, lo + cw)
                        pzc = pzs[ck][:]
                        zb = wp.tile([P, cw], dt.float32, tag="zb")
                        nc.scalar.activation(zb[:], pzc,
                                             AF.Identity,
                                             bias=consts["bcb"][:, ob:ob + 1],
                                             scale=1.0)
                        zc = wp.tile([P, cw], dt.float32, tag="zc")
                        nc.vector.tensor_tensor(zc[:], zb[:], cs[:, lsc],
                                                op=ALU.mult)
                        ot = wp.tile([P, cw], dt.float32, tag="ot")
                        nc.vector.tensor_scalar_add(
                            ot[:], zc[:], consts["bob"][:, ob:ob + 1])
                        nc.sync.dma_start(
                            out_d[ob * P:(ob + 1) * P, lsc], ot[:])
                # prefetch phase-2 x for block lb+1
                if lb + 1 < NL:
                    for hb in range(NH):
                        xbs[("p2", lb + 1, hb)] = load_xb(lb + 1, hb)

    nc.compile()
    return nc


def _get_nc():
    if "nc" not in _CACHE:
        _CACHE["nc"] = _build()
    return _CACHE["nc"]


def _prep_inputs(x, Wq, bq, Wk, bk, Wv, bv, Wo, bo):
    """Host-side precompute (fp64 for the fused weights) + per-core sharding."""
    f8 = np.float64
    M = (Wq.astype(f8).T @ Wk.astype(f8)).astype(np.float32)
    u = (Wk.astype(f8).T @ bq.astype(f8)
         + Wq.astype(f8).T @ bk.astype(f8)).astype(np.float32)
    c0 = np.float32(bq.astype(f8) @ bk.astype(f8))
    Wc = (Wo.astype(f8) @ Wv.astype(f8)).astype(np.float32)
    bc = (Wo.astype(f8) @ bv.astype(f8)).astype(np.float32)

    def _pack(WT):  # [H,H] (hin, hout) -> [NH(ob), P(hin%P), NH*P]
        t = WT.reshape(NH, P, NH, P)          # [hb, p, ob, c]
        return np.ascontiguousarray(t.transpose(2, 1, 0, 3).reshape(NH, P, NH * P))

    MT = _pack(M.T)
    WcT = _pack(Wc.T)
    ub = np.ascontiguousarray(u.reshape(NH, P).T)
    bcb = np.ascontiguousarray(bc.reshape(NH, P).T)
    bob = np.ascontiguousarray(bo.astype(np.float32).reshape(NH, P).T)
    c0b = np.full((P, 1), np.log(L - 1.0) - np.float64(c0), np.float32)
    ones = np.ones((P, P), np.float32)
    cpack = np.concatenate([ub, c0b, ones], axis=1)

    shared = dict(MT=MT, WcT=WcT, cpack=cpack, bcb=bcb, bob=bob)
    in_maps = []
    for n in range(N_CORES):
        xT = np.ascontiguousarray(x[n].astype(np.float32).T)
        in_maps.append(dict(xT=xT, **shared))
    return in_maps


def kernel(x, Wq, bq, Wk, bk, Wv, bv, Wo, bo, _trace=False, _trace_kwargs=None):
    x, Wq, bq, Wk, bk, Wv, bv, Wo, bo = (
        np.asarray(a) for a in (x, Wq, bq, Wk, bk, Wv, bv, Wo, bo))
    nc = _get_nc()
    in_maps = _prep_inputs(x, Wq, bq, Wk, bk, Wv, bv, Wo, bo)
    res = run_bass_kernel_spmd(nc, in_maps, list(range(N_CORES)),
                               trace=_trace, **(_trace_kwargs or {}))
    out = np.empty((N, L, H), np.float32)
    for n in range(N_CORES):
        out[n] = res.results[n]["outT"].T
    if _trace:
        kernel.last_result = res
    return out
